# revision 42
# baseline (speedup 1.0000x reference)
"""Fused attention kernel for Trainium2, SPMD over 8 NeuronCores.

Problem: nn_AttentionFusion (B=8, S1=S2=2048, D1=D2=512, F=256, fp32).

    Q = feat1 @ Wq + bq            [B,S1,F]
    K = feat2 @ Wk + bk            [B,S2,F]
    V = feat2 @ Wv + bv            [B,S2,F]
    A = softmax(Q K^T / sqrt(F))   [B,S1,S2]
    out = (A @ V) @ Wfc + bfc      [B,S1,F]

Sharding: pure data-parallel over batch - core i computes batch element i.

Per-core algorithm (v2):
  *  Wfc is folded into the V projection: A@V@Wfc == A@(V@Wfc), so the kernel
     precomputes Wv' = Wv@Wfc on the PE during the initial DMA lead-in (when
     the PE would otherwise idle) and never materializes attn_out - each PV
     result block is normalized, biased and DMA'd straight out. The V bias
     folds into a constant output bias: obias = bv@Wfc + bfc (A rows sum to 1).
  *  feat2 is processed first (K^T in [f,s2], V' in [s2,g] with a ones column
     for the softmax denominator), then feat1 super-block 0 -> Q^T [f,s1].
     The main loop pipelines scores^T(sup) / f1+Q(sup+1) / PV(sup-1) so the
     exp drains (ACT) and DVE drains overlap the PE stream.
  *  scores^T = K^T-chunk.T @ Q^T with exp fused into the PSUM drain (no max
     subtraction: scores ~ N(0,1), fp32-safe). P^T lands exactly in the
     layout the PV matmul needs as stationary operand; (P@V')/denom with the
     denominator from the ones column; output = psa*recip + obias in one
     fused DVE op.
  *  DMA priority: wv/wfc first on the sync queue (they gate the Wv'
     precompute), feat2 pairs + wk/wq interleaved on the gpsimd SWDGE queue
     (which casts fp32->bf16 in flight) so the first feat tiles are not
     queued behind megabytes of weights.
"""

import os
from contextlib import ExitStack

import numpy as np

import concourse.bacc as bacc
import concourse.bass as bass
import concourse.mybir as mybir
import concourse.tile as tile
from concourse.bass_utils import run_bass_kernel_spmd
from concourse.masks import make_identity

# Problem sizes (hardcoded per the harness contract).
B = 8
S = 2048          # S1 == S2
D = 512           # D1 == D2
F = 256           # fusion dim (also the output dim G of Wfc)
G = 256
N_CORES = 8
P = 128           # partitions

DC = D // P       # 4 d-chunks
FC = F // P       # 2 f-chunks
NS = S // P       # 16 s-tiles
SUPER = 512       # s1 super-block width for scores
NSUP = S // SUPER # 4 super-blocks
# Feat loads are issued just-in-time (one f2 + one f1 quad in flight): the
# DMA queues serve in-flight transfers round-robin at descriptor granularity,
# so every extra in-flight load dilutes the bandwidth of the one the PE
# needs next.

FP32 = mybir.dt.float32
BF16 = mybir.dt.bfloat16


def attention_body(ctx, tc, out, feat1, feat2, Wq, bq, Wk, bk, Wv, bv, Wfc, bfc):
    """Emit the per-core attention program.

    out:   [S, G] fp32 DRAM
    feat1: [S, D], feat2: [S, D] fp32 DRAM
    Wq/Wk/Wv: [D, F], Wfc: [F, G], biases [F]/[G] fp32 DRAM
    """
    nc = tc.nc
    Ident = mybir.ActivationFunctionType.Identity
    Exp = mybir.ActivationFunctionType.Exp
    Mult = mybir.AluOpType.mult
    Add = mybir.AluOpType.add
    scale = 1.0 / float(np.sqrt(F))

    # ---------------- pools ----------------
    consts = ctx.enter_context(tc.tile_pool(name="consts", bufs=1))
    persist = ctx.enter_context(tc.tile_pool(name="persist", bufs=1))
    ld_pool = ctx.enter_context(tc.tile_pool(name="ld", bufs=4))
    pt_pool = ctx.enter_context(tc.tile_pool(name="pt", bufs=4))
    ao_pool = ctx.enter_context(tc.tile_pool(name="ao", bufs=3))
    # PSUM: scores tiles are 2 banks x2 bufs; everything else 1 bank x4 bufs.
    ps_sc = ctx.enter_context(tc.tile_pool(name="ps_sc", bufs=2, space="PSUM"))
    ps_sm = ctx.enter_context(tc.tile_pool(name="ps_sm", bufs=4, space="PSUM"))

    # Persistent activations.
    qt_sb = persist.tile([P, FC, S], BF16)      # Q^T  [f, s1]
    kt_sb = persist.tile([P, FC, S], BF16)      # K^T  [f, s2]
    # V' = feat2 @ (Wv@Wfc), padded to G+2 cols: col G is the softmax
    # denominator ones column, col G+1 is dead padding.
    v2_sb = persist.tile([P, NS, G + 2], BF16)  # V' (+ones col) [s2, g+2]
    f1T = persist.tile([P, DC, S], BF16)        # feat1^T [d, s1]
    f2T = persist.tile([P, DC, S], BF16)        # feat2^T [d, s2]

    # gpsimd constants first so later engine waits are cheap. memset a
    # contiguous fp32 stage, DVE casts into the strided bf16 ones column.
    ones_stage = consts.tile([P, NS, 2], FP32)
    nc.gpsimd.memset(ones_stage[:], 1.0)
    nc.vector.tensor_copy(v2_sb[:, :, G:G + 2], ones_stage[:])
    ones128 = consts.tile([P, P], FP32)
    nc.gpsimd.memset(ones128[:], 1.0)

    ident = consts.tile([P, P], FP32)
    make_identity(nc, ident[:])
    ident_bf = consts.tile([P, P], BF16)
    nc.vector.tensor_copy(ident_bf[:], ident[:])


    # ---------------- DMA issue, in priority order ----------------
    # Everything sizable rides the gpsimd SWDGE queue (casts fp32->bf16 in
    # flight, no staging), ordered by when the PE needs it: f2 pair 0/1 + wk
    # first (K(0) is the first real PE work), then wv/wfc (the Wv' precompute
    # slots into the natural PE gap after K(0)), then f1 pair 0/1 + wq.
    # Feat loads use the "(p n)" row labeling: s = p*16 + n, so one QUAD
    # (4 s-tiles = one super-block, [P, 4, D]) reads 4 contiguous DRAM rows
    # per partition = 1 big descriptor each, instead of 4 scattered ones.
    # On chip the s-axis is thereby a fixed permutation of the true s; every
    # consumer (K^T/Q^T columns, V'/P^T tiles, PV outputs) uses the same
    # permutation consistently, and the output DMA un-permutes via the same
    # rearranged AP. Consumption order: block s takes f2 quad s (K+V'),
    # then f1 quad s (Q).
    feat1_r = feat1.rearrange("(p n) d -> p n d", p=P)  # [128, 16, 512]
    feat2_r = feat2.rearrange("(p n) d -> p n d", p=P)
    out_r = out.rearrange("(p n) g -> p n g", p=P)
    schedule = []
    for s in range(NSUP):
        schedule.append((feat2_r, f2T, s))
        schedule.append((feat1_r, f1T, s))
    loads = {}
    issued = [False] * len(schedule)

    def issue_load(k):
        feat_r, _, q = schedule[k]
        ft = ld_pool.tile([P, 4, D], BF16, tag="ld")
        nc.gpsimd.dma_start(ft[:], feat_r[:, 4 * q:4 * q + 4, :])
        loads[k] = ft
        issued[k] = True

    def top_up():
        for k in range(len(schedule)):
            if not issued[k]:
                issue_load(k)
                break

    # Front order: wv/wfc (gate the precompute), then block0's quads with
    # wk/wq woven in. Keep the front SHALLOW - every additional in-flight
    # stream dilutes the bandwidth of the transfer the PE needs next
    # (measured: front-loading block1's f2 pairs here cost ~4us).
    wv_sb = consts.tile([P, DC, F], BF16)
    nc.gpsimd.dma_start(wv_sb[:], Wv.rearrange("(c p) f -> p c f", p=P))
    wfc_sb = consts.tile([P, FC, G], BF16)
    nc.gpsimd.dma_start(wfc_sb[:], Wfc.rearrange("(c p) g -> p c g", p=P))
    issue_load(0)
    wk_sb = consts.tile([P, DC, F], BF16)
    nc.gpsimd.dma_start(wk_sb[:], Wk.rearrange("(c p) f -> p c f", p=P))
    issue_load(1)
    wq_sb = consts.tile([P, DC, F], BF16)
    nc.gpsimd.dma_start(wq_sb[:], Wq.rearrange("(c p) f -> p c f", p=P))

    # Biases (tiny) on the scalar queue. (Putting them on the SWDGE stream
    # was measured 10us WORSE: the extra descriptor-gen serialization plus
    # ~900 tiny descriptors wedge into the ring FIFO ahead of later pairs.)
    bq_sb = consts.tile([P, FC], FP32)
    nc.scalar.dma_start(bq_sb[:], bq.rearrange("(c p) -> p c", p=P))
    bk_sb = consts.tile([P, FC], FP32)
    nc.scalar.dma_start(bk_sb[:], bk.rearrange("(c p) -> p c", p=P))
    bv_part = consts.tile([P, FC], FP32)
    nc.scalar.dma_start(bv_part[:], bv.rearrange("(c p) -> p c", p=P))
    bfc_bc = consts.tile([P, G], FP32)
    nc.scalar.dma_start(bfc_bc[:], bfc.partition_broadcast(P))

    # ---------------- Wv' = Wv@Wfc and obias ----------------
    wvT = consts.tile([P, FC, D], BF16)     # Wv^T [f, d]
    wv2_sb = consts.tile([P, DC, G], BF16)  # Wv' [d, g]
    Mb = consts.tile([P, FC, P], BF16)
    obias_bc = consts.tile([P, G], FP32)

    def emit_wv_precompute():
        """Wv' = Wv@Wfc on the PE, plus obias = bv@Wfc + bfc replicated on
        all partitions (stationary Mb[:, fc, j] = bv[fc*128+p] is constant
        across j, so stat^T@wfc gives every output partition bv@Wfc)."""
        for fc in range(FC):
            pst = ps_sm.tile([P, D], FP32, tag="ps_sm")
            for dc in range(DC):
                nc.tensor.matmul(
                    pst[:, dc * P:(dc + 1) * P],
                    wv_sb[:, dc, fc * P:(fc + 1) * P], ident_bf[:],
                    start=True, stop=True,
                )
            nc.vector.tensor_copy(wvT[:, fc, :], pst[:])
        for dc in range(DC):
            psw = ps_sm.tile([P, G], FP32, tag="ps_sm")
            for fc in range(FC):
                nc.tensor.matmul(
                    psw[:],
                    wvT[:, fc, dc * P:(dc + 1) * P],
                    wfc_sb[:, fc, :],
                    start=(fc == 0), stop=(fc == FC - 1),
                )
            nc.vector.tensor_copy(wv2_sb[:, dc, :], psw[:])
        for fc in range(FC):
            nc.vector.tensor_scalar_mul(
                Mb[:, fc, :], ones128[:], bv_part[:, fc:fc + 1])
        ps_ob = ps_sm.tile([P, G], FP32, tag="ps_sm")
        for fc in range(FC):
            nc.tensor.matmul(
                ps_ob[:], Mb[:, fc, :], wfc_sb[:, fc, :],
                start=(fc == 0), stop=(fc == FC - 1),
            )
        nc.vector.tensor_add(obias_bc[:], ps_ob[:], bfc_bc[:])

    # ---------------- building blocks ----------------
    def run_transpose_quad(k):
        """Transpose one loaded quad (4 s-tiles x 4 d-chunks) into its
        featT tile via regular bf16 matmuls against the identity. (The xbar
        dma_start_transpose route was measured 2x SLOWER overall: the
        serialized DMA-transpose lane becomes the bottleneck.)"""
        _, fT, q = schedule[k]
        ft = loads.pop(k)
        for n in range(4):
            i = 4 * q + n
            pst = ps_sm.tile([P, D], FP32, tag="ps_sm")
            for dc in range(DC):
                nc.tensor.matmul(
                    pst[:, dc * P:(dc + 1) * P], ft[:, n, dc * P:(dc + 1) * P],
                    ident_bf[:], start=True, stop=True,
                )
            nc.vector.tensor_copy(
                fT[:, :, i * P:(i + 1) * P],
                pst[:].rearrange("p (c s) -> p c s", c=DC),
            )
        top_up()

    def emit_proj(fT, w_sb, b_sb, dst, sup):
        """Q^T/K^T for one super-block: [f, s] = W-chunk.T @ featT."""
        s_lo, s_hi = sup * SUPER, (sup + 1) * SUPER
        for fc in range(FC):
            psq = ps_sm.tile([P, SUPER], FP32, tag="ps_sm")
            for dc in range(DC):
                nc.tensor.matmul(
                    psq[:],
                    w_sb[:, dc, fc * P:(fc + 1) * P],
                    fT[:, dc, s_lo:s_hi],
                    start=(dc == 0), stop=(dc == DC - 1),
                )
            nc.scalar.activation(
                dst[:, fc, s_lo:s_hi], psq[:], Ident, bias=b_sb[:, fc:fc + 1],
            )

    def emit_v2_tile(i):
        """V' tile i: [s2-128, g] = feat2T-chunk.T @ Wv' (ACT drain)."""
        psv = ps_sm.tile([P, G], FP32, tag="ps_sm")
        for dc in range(DC):
            nc.tensor.matmul(
                psv[:],
                f2T[:, dc, i * P:(i + 1) * P],
                wv2_sb[:, dc, :],
                start=(dc == 0), stop=(dc == DC - 1),
            )
        nc.scalar.activation(v2_sb[:, i, 0:G], psv[:], Ident)

    def emit_score_group(sup, g, pt):
        """One scores^T group: s2-chunk pair (2g, 2g+1) accumulated into a
        2-bank PSUM tile, exp'd (1024 cols) straight into pt."""
        s_lo, s_hi = sup * SUPER, (sup + 1) * SUPER
        s2c = 2 * g
        pss = ps_sc.tile([P, 2, SUPER], FP32, tag="ps_sc")
        for half in range(2):
            for fc in range(FC):
                nc.tensor.matmul(
                    pss[:, half, :],
                    kt_sb[:, fc, (s2c + half) * P:(s2c + half + 1) * P],
                    qt_sb[:, fc, s_lo:s_hi],
                    start=(fc == 0), stop=(fc == FC - 1),
                )
        nc.scalar.activation(pt[:, s2c:s2c + 2, :], pss[:], Exp, scale=scale)

    def emit_pv_block(sup, b, pt):
        """PV block: psa = P^T-chunks.T @ V'_aug; col G is the softmax
        denominator. out = psa*recip + obias in one fused DVE op, then DMA."""
        psa = ps_sm.tile([P, G + 2], FP32, tag="ps_sm")
        for s2c in range(NS):
            nc.tensor.matmul(
                psa[:],
                pt[:, s2c, b * P:(b + 1) * P],
                v2_sb[:, s2c, :],
                start=(s2c == 0), stop=(s2c == NS - 1),
            )
        recip = ao_pool.tile([P, 1], FP32, tag="recip")
        nc.vector.reciprocal_approx_fast(recip[:], psa[:, G:G + 1])
        o_sb = ao_pool.tile([P, G], FP32, tag="o_sb")
        nc.vector.scalar_tensor_tensor(
            o_sb[:], psa[:, 0:G], recip[:], obias_bc[:], Mult, Add,
        )
        # Partition p of o_sb is (permuted) row s1 = p*16 + tile-index; the
        # rearranged out AP writes each row back to its true location.
        nc.sync.dma_start(out_r[:, 4 * sup + b, :], o_sb[:])

    emit_wv_precompute()

    # ---------------- main: demand-ordered blocks ----------------
    # Block s (as its 4 feat pairs arrive): K(s) + V'(s) + Q(s), then every
    # score group the new K columns unlock - (q<s, g=2s/2s+1) against older
    # Q supers plus (s, g<=2s+1). This keeps PE work unlocked per arrived
    # byte from the first block, instead of serializing proj->scores phases.
    pt_tiles = {}
    for s in range(NSUP):
        run_transpose_quad(2 * s)          # f2 quad s
        emit_proj(f2T, wk_sb, bk_sb, kt_sb, s)
        for i in range(4 * s, 4 * s + 4):
            emit_v2_tile(i)
        run_transpose_quad(2 * s + 1)      # f1 quad s
        emit_proj(f1T, wq_sb, bq_sb, qt_sb, s)
        pt_cur = pt_pool.tile([P, NS, SUPER], BF16, tag="pt")
        pt_tiles[s] = pt_cur
        for q in range(s):
            emit_score_group(q, 2 * s, pt_tiles[q])
            emit_score_group(q, 2 * s + 1, pt_tiles[q])
        for g in range(2 * s + 2):
            emit_score_group(s, g, pt_tiles[s])

    # ---------------- PV + output ----------------
    for q in range(NSUP):
        for b in range(4):
            emit_pv_block(q, b, pt_tiles[q])


def build_program():
    # Bacc (not raw Bass): its compile() legalizes semaphore waits to the
    # TRN2 one-wait-per-instruction constraint (move_matmul_waits_to_ldweights
    # + generate_event_semaphores), which walrus codegen requires.
    nc = bacc.Bacc("TRN2", target_bir_lowering=False, debug=False)
    feat1 = nc.dram_tensor("feat1", [S, D], FP32, kind="ExternalInput").ap()
    feat2 = nc.dram_tensor("feat2", [S, D], FP32, kind="ExternalInput").ap()
    Wq = nc.dram_tensor("Wq", [D, F], FP32, kind="ExternalInput").ap()
    bq = nc.dram_tensor("bq", [F], FP32, kind="ExternalInput").ap()
    Wk = nc.dram_tensor("Wk", [D, F], FP32, kind="ExternalInput").ap()
    bk = nc.dram_tensor("bk", [F], FP32, kind="ExternalInput").ap()
    Wv = nc.dram_tensor("Wv", [D, F], FP32, kind="ExternalInput").ap()
    bv = nc.dram_tensor("bv", [F], FP32, kind="ExternalInput").ap()
    Wfc = nc.dram_tensor("Wfc", [F, G], FP32, kind="ExternalInput").ap()
    bfc = nc.dram_tensor("bfc", [G], FP32, kind="ExternalInput").ap()
    out = nc.dram_tensor("out", [S, G], FP32, kind="ExternalOutput").ap()

    with tile.TileContext(nc) as tc, ExitStack() as ctx:
        attention_body(ctx, tc, out, feat1, feat2, Wq, bq, Wk, bk, Wv, bv, Wfc, bfc)
    nc.compile()
    return nc


def run(inputs, trace=False, trace_kwargs=None):
    """Shard over 8 cores, execute, gather. Returns (output, BassKernelResults)."""
    nc = build_program()
    shared = {
        k: np.ascontiguousarray(np.asarray(inputs[k], dtype=np.float32))
        for k in ("Wq", "bq", "Wk", "bk", "Wv", "bv", "Wfc", "bfc")
    }
    feat1 = np.asarray(inputs["feat1"], dtype=np.float32)
    feat2 = np.asarray(inputs["feat2"], dtype=np.float32)
    in_maps = [
        {
            "feat1": np.ascontiguousarray(feat1[i]),
            "feat2": np.ascontiguousarray(feat2[i]),
            **shared,
        }
        for i in range(N_CORES)
    ]
    res = run_bass_kernel_spmd(
        nc, in_maps, core_ids=list(range(N_CORES)),
        trace=trace, **(trace_kwargs or {}),
    )
    out = np.stack([res.results[i]["out"] for i in range(N_CORES)], axis=0)
    return out, res


def kernel(**inputs) -> np.ndarray:
    out, _ = run(inputs)
    return out


# revision 48
# speedup vs baseline: 1.2091x; 1.2091x over previous
"""Fused attention kernel for Trainium2, SPMD over 8 NeuronCores.

Problem: nn_AttentionFusion (B=8, S1=S2=2048, D1=D2=512, F=256, fp32).

    Q = feat1 @ Wq + bq            [B,S1,F]
    K = feat2 @ Wk + bk            [B,S2,F]
    V = feat2 @ Wv + bv            [B,S2,F]
    A = softmax(Q K^T / sqrt(F))   [B,S1,S2]
    out = (A @ V) @ Wfc + bfc      [B,S1,F]

Sharding: pure data-parallel over batch - core i computes batch element i.

Per-core algorithm:
  *  Wfc is folded into the V projection: A@V@Wfc == A@(V@Wfc), so the kernel
     precomputes Wv' = Wv@Wfc on the PE during the initial DMA lead-in (when
     the PE would otherwise idle) and never materializes attn_out - each PV
     result block is normalized, biased and DMA'd straight out. The V bias
     folds into a constant output bias: obias = bv@Wfc + bfc (A rows sum to 1).
  *  Demand-ordered blocks: block s transposes f2 pairs 2s,2s+1 (PE matmul
     vs identity), projects K^T(s) [f,s2] + V'(s) [s2,g] (with a ones column
     for the softmax denominator), transposes f1 pairs, projects Q^T(s), then
     emits every scores group the new K columns unlock - (q<s, g=2s,2s+1)
     plus (s, g<=2s+1). This keeps PE work unlocked per arrived DMA byte;
     the bacc list-scheduler interleaves locally. 16 PV blocks close it out.
  *  scores^T = K^T-chunk.T @ Q^T with exp fused into the PSUM drain (no max
     subtraction: scores ~ N(0,1), fp32-safe). P^T lands exactly in the
     layout the PV matmul needs as stationary operand; (P@V')/denom with the
     denominator from the ones column; output = psa*recip + obias in one
     fused DVE op.
  *  All sizable DMAs ride the gpsimd SWDGE queue (casts fp32->bf16 in
     flight), issue-ordered by PE need: wv/wfc, f2 pair 0/1 + wk, f1 pair
     0/1 + wq, rest just-in-time. Queues serve in-flight transfers
     round-robin per descriptor, so a SHALLOW front is faster for the
     critical next transfer. Biases stay on the scalar HWDGE queue.
"""

import os
from contextlib import ExitStack

import numpy as np

import concourse.bacc as bacc
import concourse.bass as bass
import concourse.mybir as mybir
import concourse.tile as tile
from concourse.bass_utils import run_bass_kernel_spmd
from concourse.masks import make_identity

# Problem sizes (hardcoded per the harness contract).
B = 8
S = 2048          # S1 == S2
D = 512           # D1 == D2
F = 256           # fusion dim (also the output dim G of Wfc)
G = 256
N_CORES = 8
P = 128           # partitions

DC = D // P       # 4 d-chunks
FC = F // P       # 2 f-chunks
NS = S // P       # 16 s-tiles
SUPER = 512       # s1 super-block width for scores
NSUP = S // SUPER # 4 super-blocks
# Feat loads are issued just-in-time (one f2 + one f1 quad in flight): the
# DMA queues serve in-flight transfers round-robin at descriptor granularity,
# so every extra in-flight load dilutes the bandwidth of the one the PE
# needs next.

FP32 = mybir.dt.float32
BF16 = mybir.dt.bfloat16


def attention_body(ctx, tc, out, feat1, feat2, Wq, bq, Wk, bk, Wv, bv, Wfc, bfc):
    """Emit the per-core attention program.

    out:   [S, G] fp32 DRAM
    feat1: [S, D], feat2: [S, D] fp32 DRAM
    Wq/Wk/Wv: [D, F], Wfc: [F, G], biases [F]/[G] fp32 DRAM
    """
    nc = tc.nc
    Ident = mybir.ActivationFunctionType.Identity
    Exp = mybir.ActivationFunctionType.Exp
    Mult = mybir.AluOpType.mult
    Add = mybir.AluOpType.add
    scale = 1.0 / float(np.sqrt(F))

    # ---------------- pools ----------------
    consts = ctx.enter_context(tc.tile_pool(name="consts", bufs=1))
    persist = ctx.enter_context(tc.tile_pool(name="persist", bufs=1))
    ld_pool = ctx.enter_context(tc.tile_pool(name="ld", bufs=6))
    pt_pool = ctx.enter_context(tc.tile_pool(name="pt", bufs=4))
    ao_pool = ctx.enter_context(tc.tile_pool(name="ao", bufs=3))
    # PSUM: scores tiles are 2 banks x2 bufs; everything else 1 bank x4 bufs.
    ps_sc = ctx.enter_context(tc.tile_pool(name="ps_sc", bufs=2, space="PSUM"))
    ps_sm = ctx.enter_context(tc.tile_pool(name="ps_sm", bufs=4, space="PSUM"))

    # Persistent activations.
    qt_sb = persist.tile([P, FC, S], BF16)      # Q^T  [f, s1]
    kt_sb = persist.tile([P, FC, S], BF16)      # K^T  [f, s2]
    # V' = feat2 @ (Wv@Wfc), padded to G+2 cols: col G is the softmax
    # denominator ones column, col G+1 is dead padding.
    v2_sb = persist.tile([P, NS, G + 2], BF16)  # V' (+ones col) [s2, g+2]
    f1T = persist.tile([P, DC, S], BF16)        # feat1^T [d, s1]
    f2T = persist.tile([P, DC, S], BF16)        # feat2^T [d, s2]

    # gpsimd constants first so later engine waits are cheap. memset a
    # contiguous fp32 stage, DVE casts into the strided bf16 ones column.
    ones_stage = consts.tile([P, NS, 2], FP32)
    nc.gpsimd.memset(ones_stage[:], 1.0)
    nc.vector.tensor_copy(v2_sb[:, :, G:G + 2], ones_stage[:])
    ones128 = consts.tile([P, P], FP32)
    nc.gpsimd.memset(ones128[:], 1.0)

    ident = consts.tile([P, P], FP32)
    make_identity(nc, ident[:])
    ident_bf = consts.tile([P, P], BF16)
    nc.vector.tensor_copy(ident_bf[:], ident[:])


    # ---------------- DMA issue, in priority order ----------------
    # Everything sizable rides the gpsimd SWDGE queue (casts fp32->bf16 in
    # flight, no staging), ordered by when the PE needs it: f2 pair 0/1 + wk
    # first (K(0) is the first real PE work), then wv/wfc (the Wv' precompute
    # slots into the natural PE gap after K(0)), then f1 pair 0/1 + wq.
    # Consumption-ordered schedule: block s consumes f2 pairs 2s,2s+1 (for
    # K(s)+V'(s)) then f1 pairs 2s,2s+1 (for Q(s)). Pair loads keep the
    # "(n p)" labeling and 2KB-row descriptors - quad loads with 16KB
    # descriptors were measured 25us WORSE (too few descriptors to spread
    # across the 16 queues).
    feat1_r = feat1.rearrange("(n p) d -> p n d", p=P)  # [128, 16, 512]
    feat2_r = feat2.rearrange("(n p) d -> p n d", p=P)
    schedule = []
    for s in range(NSUP):
        schedule.append((feat2_r, f2T, 2 * s))
        schedule.append((feat2_r, f2T, 2 * s + 1))
        schedule.append((feat1_r, f1T, 2 * s))
        schedule.append((feat1_r, f1T, 2 * s + 1))
    loads = {}
    issued = [False] * len(schedule)

    def issue_load(k):
        feat_r, _, pair = schedule[k]
        ft = ld_pool.tile([P, 2, D], BF16, tag="ld")
        nc.gpsimd.dma_start(ft[:], feat_r[:, 2 * pair:2 * pair + 2, :])
        loads[k] = ft
        issued[k] = True

    def top_up():
        for k in range(len(schedule)):
            if not issued[k]:
                issue_load(k)
                break

    # Front order: wv/wfc (gate the precompute), then block0's pairs with
    # wk/wq woven in. Keep the front SHALLOW - every additional in-flight
    # stream dilutes the bandwidth of the transfer the PE needs next
    # (measured: front-loading block1's f2 pairs here cost ~4us).
    wv_sb = consts.tile([P, DC, F], BF16)
    nc.gpsimd.dma_start(wv_sb[:], Wv.rearrange("(c p) f -> p c f", p=P))
    wfc_sb = consts.tile([P, FC, G], BF16)
    nc.gpsimd.dma_start(wfc_sb[:], Wfc.rearrange("(c p) g -> p c g", p=P))
    issue_load(0)
    issue_load(1)
    wk_sb = consts.tile([P, DC, F], BF16)
    nc.gpsimd.dma_start(wk_sb[:], Wk.rearrange("(c p) f -> p c f", p=P))
    issue_load(2)
    issue_load(3)
    wq_sb = consts.tile([P, DC, F], BF16)
    nc.gpsimd.dma_start(wq_sb[:], Wq.rearrange("(c p) f -> p c f", p=P))

    # Biases (tiny) on the scalar queue. (Putting them on the SWDGE stream
    # was measured 10us WORSE: the extra descriptor-gen serialization plus
    # ~900 tiny descriptors wedge into the ring FIFO ahead of later pairs.)
    bq_sb = consts.tile([P, FC], FP32)
    nc.scalar.dma_start(bq_sb[:], bq.rearrange("(c p) -> p c", p=P))
    bk_sb = consts.tile([P, FC], FP32)
    nc.scalar.dma_start(bk_sb[:], bk.rearrange("(c p) -> p c", p=P))
    bv_part = consts.tile([P, FC], FP32)
    nc.scalar.dma_start(bv_part[:], bv.rearrange("(c p) -> p c", p=P))
    bfc_bc = consts.tile([P, G], FP32)
    nc.scalar.dma_start(bfc_bc[:], bfc.partition_broadcast(P))

    # ---------------- Wv' = Wv@Wfc and obias ----------------
    wvT = consts.tile([P, FC, D], BF16)     # Wv^T [f, d]
    wv2_sb = consts.tile([P, DC, G], BF16)  # Wv' [d, g]
    Mb = consts.tile([P, FC, P], BF16)
    obias_bc = consts.tile([P, G], FP32)

    def emit_wv_precompute():
        """Wv' = Wv@Wfc on the PE, plus obias = bv@Wfc + bfc replicated on
        all partitions (stationary Mb[:, fc, j] = bv[fc*128+p] is constant
        across j, so stat^T@wfc gives every output partition bv@Wfc)."""
        for fc in range(FC):
            pst = ps_sm.tile([P, D], FP32, tag="ps_sm")
            for dc in range(DC):
                nc.tensor.matmul(
                    pst[:, dc * P:(dc + 1) * P],
                    wv_sb[:, dc, fc * P:(fc + 1) * P], ident_bf[:],
                    start=True, stop=True,
                )
            nc.vector.tensor_copy(wvT[:, fc, :], pst[:])
        for dc in range(DC):
            psw = ps_sm.tile([P, G], FP32, tag="ps_sm")
            for fc in range(FC):
                nc.tensor.matmul(
                    psw[:],
                    wvT[:, fc, dc * P:(dc + 1) * P],
                    wfc_sb[:, fc, :],
                    start=(fc == 0), stop=(fc == FC - 1),
                )
            nc.vector.tensor_copy(wv2_sb[:, dc, :], psw[:])
        for fc in range(FC):
            nc.vector.tensor_scalar_mul(
                Mb[:, fc, :], ones128[:], bv_part[:, fc:fc + 1])
        ps_ob = ps_sm.tile([P, G], FP32, tag="ps_sm")
        for fc in range(FC):
            nc.tensor.matmul(
                ps_ob[:], Mb[:, fc, :], wfc_sb[:, fc, :],
                start=(fc == 0), stop=(fc == FC - 1),
            )
        nc.vector.tensor_add(obias_bc[:], ps_ob[:], bfc_bc[:])

    # ---------------- building blocks ----------------
    def run_transpose_pair(k):
        """Transpose one loaded pair (2 s-tiles x 4 d-chunks) into its
        featT tile via regular bf16 matmuls against the identity. (The xbar
        dma_start_transpose route was measured 2x SLOWER overall: the
        serialized DMA-transpose lane becomes the bottleneck.)"""
        _, fT, pair = schedule[k]
        ft = loads.pop(k)
        for j in range(2):
            i = 2 * pair + j
            pst = ps_sm.tile([P, D], FP32, tag="ps_sm")
            for dc in range(DC):
                nc.tensor.matmul(
                    pst[:, dc * P:(dc + 1) * P], ft[:, j, dc * P:(dc + 1) * P],
                    ident_bf[:], start=True, stop=True,
                )
            nc.vector.tensor_copy(
                fT[:, :, i * P:(i + 1) * P],
                pst[:].rearrange("p (c s) -> p c s", c=DC),
            )
        top_up()

    def emit_proj(fT, w_sb, b_sb, dst, sup):
        """Q^T/K^T for one super-block: [f, s] = W-chunk.T @ featT."""
        s_lo, s_hi = sup * SUPER, (sup + 1) * SUPER
        for fc in range(FC):
            psq = ps_sm.tile([P, SUPER], FP32, tag="ps_sm")
            for dc in range(DC):
                nc.tensor.matmul(
                    psq[:],
                    w_sb[:, dc, fc * P:(fc + 1) * P],
                    fT[:, dc, s_lo:s_hi],
                    start=(dc == 0), stop=(dc == DC - 1),
                )
            nc.scalar.activation(
                dst[:, fc, s_lo:s_hi], psq[:], Ident, bias=b_sb[:, fc:fc + 1],
            )

    def emit_v2_tile(i):
        """V' tile i: [s2-128, g] = feat2T-chunk.T @ Wv' (ACT drain)."""
        psv = ps_sm.tile([P, G], FP32, tag="ps_sm")
        for dc in range(DC):
            nc.tensor.matmul(
                psv[:],
                f2T[:, dc, i * P:(i + 1) * P],
                wv2_sb[:, dc, :],
                start=(dc == 0), stop=(dc == DC - 1),
            )
        nc.scalar.activation(v2_sb[:, i, 0:G], psv[:], Ident)

    def emit_score_group(sup, g, pt):
        """One scores^T group: s2-chunk pair (2g, 2g+1) accumulated into a
        2-bank PSUM tile, exp'd (1024 cols) straight into pt."""
        s_lo, s_hi = sup * SUPER, (sup + 1) * SUPER
        s2c = 2 * g
        pss = ps_sc.tile([P, 2, SUPER], FP32, tag="ps_sc")
        for half in range(2):
            for fc in range(FC):
                nc.tensor.matmul(
                    pss[:, half, :],
                    kt_sb[:, fc, (s2c + half) * P:(s2c + half + 1) * P],
                    qt_sb[:, fc, s_lo:s_hi],
                    start=(fc == 0), stop=(fc == FC - 1),
                )
        nc.scalar.activation(pt[:, s2c:s2c + 2, :], pss[:], Exp, scale=scale)

    def emit_pv_block(sup, b, pt):
        """PV block: psa = P^T-chunks.T @ V'_aug; col G is the softmax
        denominator. out = psa*recip + obias in one fused DVE op, then DMA."""
        psa = ps_sm.tile([P, G + 2], FP32, tag="ps_sm")
        for s2c in range(NS):
            nc.tensor.matmul(
                psa[:],
                pt[:, s2c, b * P:(b + 1) * P],
                v2_sb[:, s2c, :],
                start=(s2c == 0), stop=(s2c == NS - 1),
            )
        recip = ao_pool.tile([P, 1], FP32, tag="recip")
        nc.vector.reciprocal_approx_fast(recip[:], psa[:, G:G + 1])
        o_sb = ao_pool.tile([P, G], FP32, tag="o_sb")
        nc.vector.scalar_tensor_tensor(
            o_sb[:], psa[:, 0:G], recip[:], obias_bc[:], Mult, Add,
        )
        blk = sup * SUPER + b * P
        nc.sync.dma_start(out[blk:blk + P, :], o_sb[:])

    emit_wv_precompute()

    # ---------------- main: demand-ordered blocks ----------------
    # Block s (as its 4 feat pairs arrive): K(s) + V'(s) + Q(s), then every
    # score group the new K columns unlock - (q<s, g=2s/2s+1) against older
    # Q supers plus (s, g<=2s+1). This keeps PE work unlocked per arrived
    # byte from the first block, instead of serializing proj->scores phases.
    pt_tiles = {}
    for s in range(NSUP):
        run_transpose_pair(4 * s)          # f2 pair 2s
        run_transpose_pair(4 * s + 1)      # f2 pair 2s+1
        emit_proj(f2T, wk_sb, bk_sb, kt_sb, s)
        for i in range(4 * s, 4 * s + 4):
            emit_v2_tile(i)
        run_transpose_pair(4 * s + 2)      # f1 pair 2s
        run_transpose_pair(4 * s + 3)      # f1 pair 2s+1
        emit_proj(f1T, wq_sb, bq_sb, qt_sb, s)
        pt_cur = pt_pool.tile([P, NS, SUPER], BF16, tag="pt")
        pt_tiles[s] = pt_cur
        for q in range(s):
            emit_score_group(q, 2 * s, pt_tiles[q])
            emit_score_group(q, 2 * s + 1, pt_tiles[q])
        for g in range(2 * s + 2):
            emit_score_group(s, g, pt_tiles[s])

    # ---------------- PV + output ----------------
    for q in range(NSUP):
        for b in range(4):
            emit_pv_block(q, b, pt_tiles[q])


def build_program():
    # Bacc (not raw Bass): its compile() legalizes semaphore waits to the
    # TRN2 one-wait-per-instruction constraint (move_matmul_waits_to_ldweights
    # + generate_event_semaphores), which walrus codegen requires.
    nc = bacc.Bacc("TRN2", target_bir_lowering=False, debug=False)
    feat1 = nc.dram_tensor("feat1", [S, D], FP32, kind="ExternalInput").ap()
    feat2 = nc.dram_tensor("feat2", [S, D], FP32, kind="ExternalInput").ap()
    Wq = nc.dram_tensor("Wq", [D, F], FP32, kind="ExternalInput").ap()
    bq = nc.dram_tensor("bq", [F], FP32, kind="ExternalInput").ap()
    Wk = nc.dram_tensor("Wk", [D, F], FP32, kind="ExternalInput").ap()
    bk = nc.dram_tensor("bk", [F], FP32, kind="ExternalInput").ap()
    Wv = nc.dram_tensor("Wv", [D, F], FP32, kind="ExternalInput").ap()
    bv = nc.dram_tensor("bv", [F], FP32, kind="ExternalInput").ap()
    Wfc = nc.dram_tensor("Wfc", [F, G], FP32, kind="ExternalInput").ap()
    bfc = nc.dram_tensor("bfc", [G], FP32, kind="ExternalInput").ap()
    out = nc.dram_tensor("out", [S, G], FP32, kind="ExternalOutput").ap()

    with tile.TileContext(nc) as tc, ExitStack() as ctx:
        attention_body(ctx, tc, out, feat1, feat2, Wq, bq, Wk, bk, Wv, bv, Wfc, bfc)
    nc.compile()
    return nc


def run(inputs, trace=False, trace_kwargs=None):
    """Shard over 8 cores, execute, gather. Returns (output, BassKernelResults)."""
    nc = build_program()
    shared = {
        k: np.ascontiguousarray(np.asarray(inputs[k], dtype=np.float32))
        for k in ("Wq", "bq", "Wk", "bk", "Wv", "bv", "Wfc", "bfc")
    }
    feat1 = np.asarray(inputs["feat1"], dtype=np.float32)
    feat2 = np.asarray(inputs["feat2"], dtype=np.float32)
    in_maps = [
        {
            "feat1": np.ascontiguousarray(feat1[i]),
            "feat2": np.ascontiguousarray(feat2[i]),
            **shared,
        }
        for i in range(N_CORES)
    ]
    res = run_bass_kernel_spmd(
        nc, in_maps, core_ids=list(range(N_CORES)),
        trace=trace, **(trace_kwargs or {}),
    )
    out = np.stack([res.results[i]["out"] for i in range(N_CORES)], axis=0)
    return out, res


def kernel(**inputs) -> np.ndarray:
    out, _ = run(inputs)
    return out
